# revision 1
# baseline (speedup 1.0000x reference)
import sys
sys.path.insert(0, "/opt/trn_rl_repo")
sys.path.insert(0, "/opt/trn_rl_repo/concourse")
"""nn_DeformableTransformerEncoderLayer_876173328776 on 8 trn2 NeuronCores.

kernel(**inputs) takes the FULL (unsharded) inputs and returns the FULL
[2, 13294, 256] float32 output.

Sharding: 8 cores = 2 batches x 4 sequence chunks of 3328 query tokens
(padded).  Each core recomputes the value tensor for its batch (replicated
across that batch's 4 cores, per the sharding hint: sampling offsets can
reach anywhere in each level's feature map), weights replicated.

The bass program is token-major ([128 tokens, feat] tiles): PE transposes
activations for matmuls; bilinear sampling fetches 2-pixel rows (64 f32)
per (token, head, level, point, y-row) with gpsimd indirect DMA from a
head-major value buffer value[h, s, ch] in DRAM; edge handling folds the
grid_sample zero-padding rules into per-row weights; DVE does the weighted
combine + segmented reduction; FFN/LayerNorms fused on PE/ACT/DVE.
"""


import sys

sys.path.insert(0, "/opt/trn_rl_repo")
sys.path.insert(0, "/opt/trn_rl_repo/concourse")

import concourse.mybir as mybir

MAX_WAITS_PER_INST = 1


def split_waits(nc):
    """Post-process all functions of `nc` so no instruction carries more than
    MAX_WAITS_PER_INST sem-waits."""
    n_split = 0
    for fn in nc.m.functions:
        for bb in fn.blocks:
            insts = bb.instructions
            i = 0
            while i < len(insts):
                inst = insts[i]
                si = inst.sync_info
                if si is None or si.on_wait is None or len(si.on_wait) <= MAX_WAITS_PER_INST:
                    i += 1
                    continue
                waits = list(si.on_wait)
                si.on_wait = waits[-MAX_WAITS_PER_INST:]
                rest = waits[:-MAX_WAITS_PER_INST]
                nops = []
                for k in range(0, len(rest), MAX_WAITS_PER_INST):
                    nop = mybir.InstNoOp(
                        name=nc.get_next_instruction_name(),
                        engine=inst.engine,
                        ins=[],
                        outs=[],
                        sync_info=mybir.SyncInfo(
                            on_wait=rest[k : k + MAX_WAITS_PER_INST], on_update=[]
                        ),
                        text_hint="wait_split",
                    )
                    nc.register_instruction(nop, overwrite=True)
                    nops.append(nop)
                insts[i:i] = nops
                i += len(nops) + 1
                n_split += 1
    return n_split


def apply():
    """Monkeypatch TileContext.__exit__ to run split_waits after scheduling."""
    import concourse.tile as tile

    if getattr(tile.TileContext, "_wait_split_patched", False):
        return
    orig_exit = tile.TileContext.__exit__

    def patched_exit(self, exc_type, exc_value, tb):
        r = orig_exit(self, exc_type, exc_value, tb)
        if exc_type is None:
            split_waits(self.nc)
        return r

    tile.TileContext.__exit__ = patched_exit
    tile.TileContext._wait_split_patched = True



import time

sys.path.insert(0, "/opt/trn_rl_repo")
sys.path.insert(0, "/opt/trn_rl_repo/concourse")

import numpy as np
import jax
from jax.sharding import Mesh, PartitionSpec
from jax.experimental.shard_map import shard_map

import concourse.bass as bass
import concourse.mybir as mybir
from concourse import bass2jax
from concourse.bass2jax import _bass_exec_p, install_neuronx_cc_hook, partition_id_tensor


def _install_loud_hook():
    """neuronx_cc hook that prints the real traceback on failure (the PJRT
    layer otherwise swallows it into an opaque INTERNAL error)."""
    import traceback
    import libneuronxla

    if getattr(libneuronxla, "_loud_hook_installed", False):
        return

    def hook(*a, **k):
        try:
            return bass2jax.neuronx_cc_hook(*a, **k)
        except BaseException:
            traceback.print_exc()
            raise

    if not hasattr(libneuronxla, "orig_neuronx_cc"):
        libneuronxla.orig_neuronx_cc = libneuronxla.neuronx_cc
    libneuronxla.neuronx_cc = hook
    libneuronxla._loud_hook_installed = True


class CompiledKernel:
    def __init__(self, nc: bass.Bass, n_cores: int):
        install_neuronx_cc_hook()
        _install_loud_hook()
        assert nc.dbg_addr is None or not nc.dbg_callbacks
        self.nc = nc
        self.n_cores = n_cores
        partition_name = nc.partition_id_tensor.name if nc.partition_id_tensor else None
        in_names, out_names, out_avals, zero_outs = [], [], [], []
        for alloc in nc.m.functions[0].allocations:
            if not isinstance(alloc, mybir.MemoryLocationSet):
                continue
            name = alloc.memorylocations[0].name
            if alloc.kind == "ExternalInput":
                if name != partition_name:
                    in_names.append(name)
            elif alloc.kind == "ExternalOutput":
                shape = tuple(alloc.tensor_shape)
                dtype = mybir.dt.np(alloc.dtype)
                out_names.append(name)
                out_avals.append(jax.core.ShapedArray(shape, dtype))
                zero_outs.append(np.zeros(shape, dtype))
        n_params = len(in_names)
        n_outs = len(out_avals)
        all_in_names = list(in_names) + list(out_names)
        if partition_name is not None:
            all_in_names.append(partition_name)
        self.in_names = in_names
        self.out_names = out_names
        self.out_avals = out_avals
        self.n_params = n_params

        def _body(*args):
            operands = list(args)
            if partition_name is not None:
                operands.append(partition_id_tensor())
            outs = _bass_exec_p.bind(
                *operands,
                out_avals=tuple(out_avals),
                in_names=tuple(all_in_names),
                out_names=tuple(out_names),
                lowering_input_output_aliases=(),
                sim_require_finite=False,
                sim_require_nnan=False,
                nc=nc,
            )
            return tuple(outs)

        donate = tuple(range(n_params, n_params + n_outs))
        if n_cores == 1:
            self._fn = jax.jit(_body, donate_argnums=donate, keep_unused=True)
            self.mesh = None
        else:
            devices = jax.devices()[:n_cores]
            self.mesh = Mesh(np.asarray(devices), ("core",))
            in_specs = (PartitionSpec("core"),) * (n_params + n_outs)
            out_specs = (PartitionSpec("core"),) * n_outs
            self._fn = jax.jit(
                shard_map(_body, mesh=self.mesh, in_specs=in_specs,
                          out_specs=out_specs, check_rep=False),
                donate_argnums=donate,
                keep_unused=True,
            )
        self._zero_outs = zero_outs

    def _prep(self, in_maps):
        assert len(in_maps) == self.n_cores
        if self.n_cores == 1:
            return [np.ascontiguousarray(in_maps[0][n]) for n in self.in_names]
        cat = []
        for i, n in enumerate(self.in_names):
            cat.append(np.concatenate(
                [np.ascontiguousarray(in_maps[c][n]) for c in range(self.n_cores)], axis=0))
        return cat

    def put(self, in_maps):
        """device_put inputs once; returns opaque handle for run()."""
        arrs = self._prep(in_maps)
        if self.n_cores == 1:
            return [jax.device_put(a, jax.devices()[0]) for a in arrs]
        from jax.sharding import NamedSharding
        sh = NamedSharding(self.mesh, PartitionSpec("core"))
        return [jax.device_put(a, sh) for a in arrs]

    def _zeros(self):
        if self.n_cores == 1:
            return [np.zeros(z.shape, z.dtype) for z in self._zero_outs]
        return [np.zeros((self.n_cores * z.shape[0], *z.shape[1:]), z.dtype)
                for z in self._zero_outs]

    def run(self, handle):
        outs = self._fn(*handle, *self._zeros())
        outs = [np.asarray(o) for o in outs]
        if self.n_cores == 1:
            return [dict(zip(self.out_names, outs))]
        res = []
        for c in range(self.n_cores):
            d = {}
            for i, n in enumerate(self.out_names):
                d[n] = outs[i].reshape(self.n_cores, *self.out_avals[i].shape)[c]
            res.append(d)
        return res

    def bench(self, handle, iters=5, warmup=2):
        for _ in range(warmup):
            outs = self._fn(*handle, *self._zeros())
            jax.block_until_ready(outs)
        ts = []
        for _ in range(iters):
            t0 = time.perf_counter()
            outs = self._fn(*handle, *self._zeros())
            jax.block_until_ready(outs)
            ts.append(time.perf_counter() - t0)
        return min(ts), ts


import sys

sys.path.insert(0, "/opt/trn_rl_repo")
sys.path.insert(0, "/opt/trn_rl_repo/concourse")

import numpy as np
apply()

import concourse.bass as bass
import concourse.mybir as mybir
import concourse.tile as tile

F32 = mybir.dt.float32
I32 = mybir.dt.int32
AX = mybir.AxisListType
OP = mybir.AluOpType
AF = mybir.ActivationFunctionType

SHAPES = [(100, 100), (50, 50), (25, 25), (13, 13)]
NH, NL, NP, C, DF, HD = 8, 4, 4, 256, 1024, 32
S = 13294
SPAD = 13312          # padded batch tokens (104 * 128)
T = 3328              # per-core query tokens (26 * 128)
NBLK_A = SPAD // 128  # 104
NBLK_B = T // 128     # 26
LVL_START = [0, 10000, 12500, 13125]


def build(nc: bass.Bass):
    # ---------------- I/O ----------------
    srcb = nc.dram_tensor("srcb", [SPAD, C], F32, kind="ExternalInput")
    srcq = nc.dram_tensor("srcq", [T, C], F32, kind="ExternalInput")
    posq = nc.dram_tensor("posq", [T, C], F32, kind="ExternalInput")
    refq = nc.dram_tensor("refq", [T, 8], F32, kind="ExternalInput")
    wq = nc.dram_tensor("wq", [C, 384], F32, kind="ExternalInput")
    bq = nc.dram_tensor("bq", [1, 384], F32, kind="ExternalInput")
    wval = nc.dram_tensor("wval", [C, C], F32, kind="ExternalInput")
    bval = nc.dram_tensor("bval", [1, C], F32, kind="ExternalInput")
    wout = nc.dram_tensor("wout", [C, C], F32, kind="ExternalInput")
    bout = nc.dram_tensor("bout", [1, C], F32, kind="ExternalInput")
    w1 = nc.dram_tensor("w1", [C, DF], F32, kind="ExternalInput")
    b1t = nc.dram_tensor("b1t", [128, 8], F32, kind="ExternalInput")
    w2 = nc.dram_tensor("w2", [DF, C], F32, kind="ExternalInput")
    b2 = nc.dram_tensor("b2", [1, C], F32, kind="ExternalInput")
    gb = nc.dram_tensor("gb", [4, C], F32, kind="ExternalInput")
    # consts rows (each [128] in (h,l,p) order unless noted):
    # 0: W_l  1: W_l-2  2: H_l-1  3: H_l-2  4: h*SPAD+lvl_start  5: ones
    # 6: (l,xy)-interleaved scale row [W0,H0,W1,H1,...] (first 8 cols)
    consts = nc.dram_tensor("consts", [8, 128], F32, kind="ExternalInput")
    ident = nc.dram_tensor("ident", [128, 128], F32, kind="ExternalInput")
    out = nc.dram_tensor("out", [T, C], F32, kind="ExternalOutput")

    value_d = nc.dram_tensor("value_d", [NH, SPAD, HD], F32, kind="Internal")
    value_flat = value_d[:].rearrange("h s c -> (h s) c")

    with tile.TileContext(nc) as tc:
        # ------------- persistent weights/constants -------------
        with tc.tile_pool(name="wpool", bufs=1) as wp:
            wq_sb = [wp.tile([128, 384], F32, name=f"wq{k}", tag=f"wq{k}") for k in range(2)]
            wval_sb = [wp.tile([128, C], F32, name=f"wval{k}", tag=f"wval{k}") for k in range(2)]
            wout_sb = [wp.tile([128, C], F32, name=f"wout{k}", tag=f"wout{k}") for k in range(2)]
            w1_sb = [wp.tile([128, DF], F32, name=f"w1{k}", tag=f"w1{k}") for k in range(2)]
            w2_sb = [wp.tile([128, C], F32, name=f"w2{k}", tag=f"w2{k}") for k in range(8)]
            for k in range(2):
                nc.sync.dma_start(wq_sb[k][:], wq[128 * k:128 * (k + 1), :])
                nc.sync.dma_start(wval_sb[k][:], wval[128 * k:128 * (k + 1), :])
                nc.sync.dma_start(wout_sb[k][:], wout[128 * k:128 * (k + 1), :])
                nc.sync.dma_start(w1_sb[k][:], w1[128 * k:128 * (k + 1), :])
            for k in range(8):
                nc.sync.dma_start(w2_sb[k][:], w2[128 * k:128 * (k + 1), :])
            bq_sb = wp.tile([1, 384], F32)
            bval_sb = wp.tile([1, C], F32)
            bout_sb = wp.tile([1, C], F32)
            b2_sb = wp.tile([1, C], F32)
            b1t_sb = wp.tile([128, 8], F32)
            nc.sync.dma_start(bq_sb[:], bq[:])
            nc.sync.dma_start(bval_sb[:], bval[:])
            nc.sync.dma_start(bout_sb[:], bout[:])
            nc.sync.dma_start(b2_sb[:], b2[:])
            nc.sync.dma_start(b1t_sb[:], b1t[:])
            gbt = [wp.tile([128, C], F32, name=f"gb{i}", tag=f"gb{i}") for i in range(4)]
            for i in range(4):
                nc.sync.dma_start(gbt[i][:], gb[i:i + 1, :].to_broadcast([128, C]))
            ident_sb = wp.tile([128, 128], F32)
            nc.sync.dma_start(ident_sb[:], ident[:])
            ones_sb = wp.tile([1, 128], F32)
            nc.sync.dma_start(ones_sb[:], consts[5:6, :])
            # broadcast const rows to [128,128]
            cWl = wp.tile([128, 128], F32)
            cWm2 = wp.tile([128, 128], F32)
            cHm1 = wp.tile([128, 128], F32)
            cHm2 = wp.tile([128, 128], F32)
            cBase = wp.tile([128, 128], F32)
            for t_, row in ((cWl, 0), (cWm2, 1), (cHm1, 2), (cHm2, 3), (cBase, 4)):
                nc.sync.dma_start(t_[:], consts[row:row + 1, :].to_broadcast([128, 128]))
            cScale = wp.tile([128, 8], F32)
            nc.sync.dma_start(cScale[:], consts[6:7, 0:8].to_broadcast([128, 8]))

            # ---------------- Phase A: value ----------------
            with tc.tile_pool(name="pa", bufs=3) as pa, \
                 tc.tile_pool(name="pa_ps", bufs=2, space="PSUM") as pa_ps:
                for b in range(NBLK_A):
                    sb_s = pa.tile([128, C], F32, name="sb_s", tag="sb_s")
                    nc.sync.dma_start(sb_s[:], srcb[b * 128:(b + 1) * 128, :])
                    sT = [pa.tile([128, 128], F32, name=f"sT{k}", tag=f"sT{k}") for k in range(2)]
                    for k in range(2):
                        ps_t = pa_ps.tile([128, 128], F32, name="ps_t", tag="ps_t")
                        nc.tensor.transpose(ps_t[:], sb_s[:, 128 * k:128 * (k + 1)],
                                            ident_sb[:])
                        nc.vector.tensor_copy(sT[k][:], ps_t[:])
                    ps_v = pa_ps.tile([128, C], F32, name="ps_v", tag="ps_v")
                    nc.tensor.matmul(ps_v[:], sT[0][:], wval_sb[0][:],
                                     start=True, stop=False)
                    nc.tensor.matmul(ps_v[:], sT[1][:], wval_sb[1][:],
                                     start=False, stop=False)
                    nc.tensor.matmul(ps_v[:], ones_sb[:], bval_sb[:],
                                     start=False, stop=True)
                    sb_v = pa.tile([128, C], F32, name="sb_v", tag="sb_v")
                    nc.scalar.copy(sb_v[:], ps_v[:])
                    nc.sync.dma_start(
                        value_d[:, b * 128:(b + 1) * 128, :].rearrange("h s c -> s h c"),
                        sb_v[:])

            # ---------------- Phase B ----------------
            with tc.tile_pool(name="pb", bufs=2) as pb, \
                 tc.tile_pool(name="sc", bufs=2) as sc, \
                 tc.tile_pool(name="gat", bufs=3) as gat, \
                 tc.tile_pool(name="pb_ps", bufs=2, space="PSUM") as pbps:
                for b in range(NBLK_B):
                    tok = slice(b * 128, (b + 1) * 128)
                    sb_src = pb.tile([128, C], F32, name="sb_src", tag="sb_src")
                    sb_q = pb.tile([128, C], F32, name="sb_q", tag="sb_q")
                    sb_ref = pb.tile([128, 8], F32, name="sb_ref", tag="sb_ref")
                    nc.sync.dma_start(sb_src[:], srcq[tok, :])
                    nc.sync.dma_start(sb_q[:], posq[tok, :])
                    nc.sync.dma_start(sb_ref[:], refq[tok, :])
                    nc.vector.tensor_add(sb_q[:], sb_q[:], sb_src[:])
                    # q^T
                    qT = [pb.tile([128, 128], F32, name=f"qT{k}", tag=f"qT{k}") for k in range(2)]
                    for k in range(2):
                        ps_t = pbps.tile([128, 128], F32, name="ps_t", tag="ps_t")
                        nc.tensor.transpose(ps_t[:], sb_q[:, 128 * k:128 * (k + 1)],
                                            ident_sb[:])
                        nc.vector.tensor_copy(qT[k][:], ps_t[:])
                    # off|logits
                    ps_q = pbps.tile([128, 384], F32, name="ps_q", tag="ps_q", bufs=1)
                    nc.tensor.matmul(ps_q[:], qT[0][:], wq_sb[0][:],
                                     start=True, stop=False)
                    nc.tensor.matmul(ps_q[:], qT[1][:], wq_sb[1][:],
                                     start=False, stop=False)
                    nc.tensor.matmul(ps_q[:], ones_sb[:], bq_sb[:],
                                     start=False, stop=True)

                    # softmax over 16 per head  (logits at [:,256:384], (h,l,p))
                    logit = ps_q[:, 256:384].rearrange("p (h s) -> p h s", h=NH)
                    rmax = sc.tile([128, 8], F32, name="rmax", tag="rmax")
                    nc.vector.reduce_max(rmax[:], logit, axis=AX.X)
                    sb_e = sc.tile([128, 128], F32, name="sb_e", tag="sb_e")
                    nc.vector.scalar_tensor_tensor(
                        out=sb_e[:].rearrange("p (h s) -> p h s", h=NH),
                        in0=logit, scalar=1.0,
                        in1=rmax[:].to_broadcast([128, NH, 16]),
                        op0=OP.bypass, op1=OP.subtract)
                    nc.scalar.activation(sb_e[:], sb_e[:], AF.Exp)
                    rsum = sc.tile([128, 8], F32, name="rsum", tag="rsum")
                    nc.vector.reduce_sum(rsum[:], sb_e[:].rearrange(
                        "p (h s) -> p h s", h=NH), axis=AX.X)
                    nc.vector.reciprocal(rsum[:], rsum[:])
                    sb_aw = sc.tile([128, 128], F32, name="sb_aw", tag="sb_aw")
                    nc.vector.tensor_mul(
                        sb_aw[:].rearrange("p (h s) -> p h s", h=NH),
                        sb_e[:].rearrange("p (h s) -> p h s", h=NH),
                        rsum[:].to_broadcast([128, NH, 16]))
                    # view of aw in (l,h,p) order (sampling layout)
                    aw_lhp = sb_aw[:].rearrange("p (h l pp) -> p l h pp",
                                                h=NH, l=NL)

                    # ---- sampling coordinates ----  (all (l,h,p) order)
                    # refs_scaled = ref*(W,H) - 0.5   [128, 8] (l,xy)
                    rs = sc.tile([128, 8], F32, name="rs", tag="rs")
                    nc.vector.tensor_mul(rs[:], sb_ref[:], cScale[:])
                    nc.vector.tensor_scalar_add(rs[:], rs[:], -0.5)
                    rsv = rs[:].rearrange("p (l two) -> p l two", l=NL)
                    offv = ps_q[:, 0:256].rearrange(
                        "p (l h pp two) -> p l h pp two", h=NH, l=NL, pp=NP)
                    X = sc.tile([128, 128], F32, name="X", tag="X")
                    Y = sc.tile([128, 128], F32, name="Y", tag="Y")
                    Xv = X[:].rearrange("p (l h pp) -> p l h pp", h=NH, l=NL)
                    Yv = Y[:].rearrange("p (l h pp) -> p l h pp", h=NH, l=NL)
                    nc.vector.tensor_add(
                        Xv, offv[:, :, :, :, 0],
                        rsv[:, :, 0].to_broadcast([128, NL, NH, NP]))
                    nc.vector.tensor_add(
                        Yv, offv[:, :, :, :, 1],
                        rsv[:, :, 1].to_broadcast([128, NL, NH, NP]))

                    def floor_(dst_tag, src):
                        ti = sc.tile([128, 128], I32, tag=dst_tag + "i")
                        tf = sc.tile([128, 128], F32, tag=dst_tag)
                        nc.vector.tensor_copy(ti[:], src[:])
                        nc.vector.tensor_copy(tf[:], ti[:])
                        gt = sc.tile([128, 128], F32, tag=dst_tag + "g")
                        nc.vector.tensor_tensor(gt[:], tf[:], src[:], op=OP.is_gt)
                        nc.vector.tensor_tensor(tf[:], tf[:], gt[:], op=OP.subtract)
                        return tf

                    X0 = floor_("X0", X)
                    Y0 = floor_("Y0", Y)
                    fx = sc.tile([128, 128], F32, name="fx", tag="fx")
                    fy = sc.tile([128, 128], F32, name="fy", tag="fy")
                    nc.vector.tensor_tensor(fx[:], X[:], X0[:], op=OP.subtract)
                    nc.vector.tensor_tensor(fy[:], Y[:], Y0[:], op=OP.subtract)
                    # xs = clip(X0, 0, W-2); d = xs - X0
                    xs = sc.tile([128, 128], F32, name="xs", tag="xs")
                    nc.vector.tensor_scalar_max(xs[:], X0[:], 0.0)
                    nc.vector.tensor_tensor(xs[:], xs[:], cWm2[:], op=OP.min)
                    d = sc.tile([128, 128], F32, name="d", tag="d")
                    nc.vector.tensor_tensor(d[:], xs[:], X0[:], op=OP.subtract)
                    eq0 = sc.tile([128, 128], F32, name="eq0", tag="eq0")
                    eq1 = sc.tile([128, 128], F32, name="eq1", tag="eq1")
                    eqm1 = sc.tile([128, 128], F32, name="eqm1", tag="eqm1")
                    nc.vector.tensor_scalar(eq0[:], d[:], 0.0, None, op0=OP.is_equal)
                    nc.vector.tensor_scalar(eq1[:], d[:], 1.0, None, op0=OP.is_equal)
                    nc.vector.tensor_scalar(eqm1[:], d[:], -1.0, None, op0=OP.is_equal)
                    wx0 = sc.tile([128, 128], F32, name="wx0", tag="wx0")
                    nc.vector.tensor_scalar(wx0[:], fx[:], -1.0, 1.0,
                                            op0=OP.mult, op1=OP.add)
                    wA = sc.tile([128, 128], F32, name="wA", tag="wA")
                    wB = sc.tile([128, 128], F32, name="wB", tag="wB")
                    tt = sc.tile([128, 128], F32, name="tt", tag="tt")
                    nc.vector.tensor_mul(wA[:], wx0[:], eq0[:])
                    nc.vector.tensor_mul(tt[:], fx[:], eq1[:])
                    nc.vector.tensor_add(wA[:], wA[:], tt[:])
                    nc.vector.tensor_mul(wB[:], wx0[:], eqm1[:])
                    nc.vector.tensor_mul(tt[:], fx[:], eq0[:])
                    nc.vector.tensor_add(wB[:], wB[:], tt[:])
                    # y rows
                    ys0 = sc.tile([128, 128], F32, name="ys0", tag="ys0")
                    ys1 = sc.tile([128, 128], F32, name="ys1", tag="ys1")
                    nc.vector.tensor_scalar_max(ys0[:], Y0[:], 0.0)
                    nc.vector.tensor_tensor(ys0[:], ys0[:], cHm1[:], op=OP.min)
                    nc.vector.tensor_scalar(ys1[:], Y0[:], 1.0, 0.0,
                                            op0=OP.add, op1=OP.max)
                    nc.vector.tensor_tensor(ys1[:], ys1[:], cHm1[:], op=OP.min)
                    vy0 = sc.tile([128, 128], F32, name="vy0", tag="vy0")
                    vy1 = sc.tile([128, 128], F32, name="vy1", tag="vy1")
                    nc.vector.tensor_scalar(vy0[:], Y0[:], 0.0, None, op0=OP.is_ge)
                    nc.vector.tensor_tensor(tt[:], Y0[:], cHm1[:], op=OP.is_le)
                    nc.vector.tensor_mul(vy0[:], vy0[:], tt[:])
                    nc.vector.tensor_scalar(vy1[:], Y0[:], -1.0, None, op0=OP.is_ge)
                    nc.vector.tensor_tensor(tt[:], Y0[:], cHm2[:], op=OP.is_le)
                    nc.vector.tensor_mul(vy1[:], vy1[:], tt[:])
                    wy0 = sc.tile([128, 128], F32, name="wy0", tag="wy0")
                    wy1 = sc.tile([128, 128], F32, name="wy1", tag="wy1")
                    nc.vector.tensor_scalar(wy0[:], fy[:], -1.0, 1.0,
                                            op0=OP.mult, op1=OP.add)
                    nc.vector.tensor_mul(wy0[:], wy0[:], vy0[:])
                    nc.vector.tensor_mul(wy1[:], fy[:], vy1[:])
                    awy0 = sc.tile([128, 128], F32, name="awy0", tag="awy0")
                    awy1 = sc.tile([128, 128], F32, name="awy1", tag="awy1")
                    lhp = lambda ap: ap.rearrange("p (l h pp) -> p l h pp",
                                                  h=NH, l=NL)
                    nc.vector.tensor_mul(lhp(awy0[:]), aw_lhp, lhp(wy0[:]))
                    nc.vector.tensor_mul(lhp(awy1[:]), aw_lhp, lhp(wy1[:]))
                    # W assembly [128, (l h p r x)=512]
                    Wt = sc.tile([128, 512], F32, name="Wt", tag="Wt")
                    Wv = Wt[:].rearrange("p (q r x) -> p q r x", r=2, x=2)
                    nc.vector.tensor_mul(Wv[:, :, 0, 0], awy0[:], wA[:])
                    nc.vector.tensor_mul(Wv[:, :, 0, 1], awy0[:], wB[:])
                    nc.vector.tensor_mul(Wv[:, :, 1, 0], awy1[:], wA[:])
                    nc.vector.tensor_mul(Wv[:, :, 1, 1], awy1[:], wB[:])
                    # indices [128, (h l p r)=256] f32 -> int32
                    tb = sc.tile([128, 128], F32, name="tb", tag="tb")
                    nc.vector.tensor_add(tb[:], xs[:], cBase[:])
                    idxf = sc.tile([128, 256], F32, name="idxf", tag="idxf")
                    idxfv = idxf[:].rearrange("p (q r) -> p q r", r=2)
                    nc.vector.tensor_mul(tt[:], ys0[:], cWl[:])
                    nc.vector.tensor_add(idxfv[:, :, 0], tt[:], tb[:])
                    nc.vector.tensor_mul(tt[:], ys1[:], cWl[:])
                    nc.vector.tensor_add(idxfv[:, :, 1], tt[:], tb[:])
                    idxi = sc.tile([128, 256], I32, name="idxi", tag="idxi")
                    nc.vector.tensor_copy(idxi[:], idxf[:])

                    # ---- gather + combine per level ----
                    red = [sc.tile([128, 256], F32, name=f"red{l}", tag=f"red{l}") for l in range(NL)]
                    for l in range(NL):
                        vt = gat.tile([128, 64, 64], F32, name="vt", tag="vt")
                        # HW only honors one index per partition per call
                        for j in range(64):
                            nc.gpsimd.indirect_dma_start(
                                out=vt[:, j, :], out_offset=None, in_=value_flat,
                                in_offset=bass.IndirectOffsetOnAxis(
                                    ap=idxi[:, l * 64 + j:l * 64 + j + 1], axis=0))
                        wv = gat.tile([128, 4096], F32, name="wv", tag="wv")
                        nc.vector.tensor_mul(
                            wv[:].rearrange("p (h j x c) -> p h j x c",
                                            h=NH, j=8, x=2),
                            vt[:].rearrange("p (h j) (x c) -> p h j x c",
                                            h=NH, x=2),
                            Wt[:, l * 128:(l + 1) * 128].rearrange(
                                "p (h j x) -> p h j x", h=NH, j=8)
                            .to_broadcast([128, NH, 8, 2, HD]))
                        nc.vector.reduce_sum(
                            red[l][:].rearrange("p (h c) -> p h c", h=NH),
                            wv[:].rearrange("p (h j x c) -> p h c j x",
                                            h=NH, j=8, x=2),
                            axis=AX.XY)
                    attn = sc.tile([128, 256], F32, name="attn", tag="attn")
                    nc.vector.tensor_add(tt[:], red[0][:, 0:128], red[1][:, 0:128])
                    nc.vector.tensor_add(attn[:, 0:128], tt[:], red[2][:, 0:128])
                    nc.vector.tensor_add(attn[:, 0:128], attn[:, 0:128],
                                         red[3][:, 0:128])
                    nc.vector.tensor_add(tt[:], red[0][:, 128:256], red[1][:, 128:256])
                    nc.vector.tensor_add(attn[:, 128:256], tt[:], red[2][:, 128:256])
                    nc.vector.tensor_add(attn[:, 128:256], attn[:, 128:256],
                                         red[3][:, 128:256])

                    # ---- src2 = attn @ Wout + bout ----
                    aT = [pb.tile([128, 128], F32, name=f"aT{k}", tag=f"aT{k}") for k in range(2)]
                    for k in range(2):
                        ps_t = pbps.tile([128, 128], F32, name="ps_t", tag="ps_t")
                        nc.tensor.transpose(ps_t[:], attn[:, 128 * k:128 * (k + 1)],
                                            ident_sb[:])
                        nc.vector.tensor_copy(aT[k][:], ps_t[:])
                    ps_o = pbps.tile([128, C], F32, name="ps_o", tag="ps_o", bufs=1)
                    nc.tensor.matmul(ps_o[:], aT[0][:], wout_sb[0][:],
                                     start=True, stop=False)
                    nc.tensor.matmul(ps_o[:], aT[1][:], wout_sb[1][:],
                                     start=False, stop=False)
                    nc.tensor.matmul(ps_o[:], ones_sb[:], bout_sb[:],
                                     start=False, stop=True)

                    # ---- x = LN1(src + src2) ----
                    def layer_norm(ps_in, resid, g_t, b_t, out_tile):
                        r = sc.tile([128, C], F32, name="ln_r", tag="ln_r")
                        msum = sc.tile([128, 1], F32, name="ln_m", tag="ln_m")
                        nc.vector.scalar_tensor_tensor(
                            out=r[:], in0=ps_in, scalar=1.0, in1=resid,
                            op0=OP.bypass, op1=OP.add, accum_out=msum[:])
                        nc.vector.tensor_scalar_mul(msum[:], msum[:], -1.0 / C)
                        xc = sc.tile([128, C], F32, name="ln_xc", tag="ln_xc")
                        nc.vector.tensor_scalar_add(xc[:], r[:], msum[:])
                        sq = sc.tile([128, C], F32, name="ln_sq", tag="ln_sq")
                        vsum = sc.tile([128, 1], F32, name="ln_v", tag="ln_v")
                        nc.scalar.activation(sq[:], xc[:], AF.Square,
                                             accum_out=vsum[:])
                        nc.vector.tensor_scalar(vsum[:], vsum[:], 1.0 / C, 1e-5,
                                                op0=OP.mult, op1=OP.add)
                        nc.scalar.sqrt(vsum[:], vsum[:])
                        nc.vector.reciprocal(vsum[:], vsum[:])
                        nc.vector.scalar_tensor_tensor(
                            out=out_tile, in0=xc[:], scalar=vsum[:], in1=g_t,
                            op0=OP.mult, op1=OP.mult)
                        nc.vector.tensor_add(out_tile, out_tile, b_t)
                        return out_tile

                    sb_x = pb.tile([128, C], F32, name="sb_x", tag="sb_x")
                    layer_norm(ps_o[:], sb_src[:], gbt[0][:], gbt[1][:], sb_x[:])

                    # ---- FFN ----
                    xT = [pb.tile([128, 128], F32, name=f"xT{k}", tag=f"xT{k}") for k in range(2)]
                    for k in range(2):
                        ps_t = pbps.tile([128, 128], F32, name="ps_t", tag="ps_t")
                        nc.tensor.transpose(ps_t[:], sb_x[:, 128 * k:128 * (k + 1)],
                                            ident_sb[:])
                        nc.vector.tensor_copy(xT[k][:], ps_t[:])
                    h1 = [pb.tile([128, 128], F32, name=f"h1_{m}", tag=f"h1_{m}") for m in range(8)]
                    for m in range(8):
                        ps_h = pbps.tile([128, 128], F32, name="ps_h", tag="ps_h")
                        nc.tensor.matmul(ps_h[:], w1_sb[0][:, 128 * m:128 * (m + 1)],
                                         xT[0][:], start=True, stop=False)
                        nc.tensor.matmul(ps_h[:], w1_sb[1][:, 128 * m:128 * (m + 1)],
                                         xT[1][:], start=False, stop=True)
                        nc.scalar.activation(h1[m][:], ps_h[:], AF.Relu,
                                             bias=b1t_sb[:, m:m + 1])
                    ps_f = pbps.tile([128, C], F32, name="ps_f", tag="ps_f", bufs=1)
                    for m in range(8):
                        nc.tensor.matmul(ps_f[:], h1[m][:], w2_sb[m][:],
                                         start=(m == 0), stop=False)
                    nc.tensor.matmul(ps_f[:], ones_sb[:], b2_sb[:],
                                     start=False, stop=True)
                    sb_out = pb.tile([128, C], F32, name="sb_out", tag="sb_out")
                    layer_norm(ps_f[:], sb_x[:], gbt[2][:], gbt[3][:], sb_out[:])
                    nc.sync.dma_start(out[tok, :], sb_out[:])
    return nc


def make_host_inputs(src_b, srcq, posq, refq, inputs):
    """Build the per-core input map (numpy) given the batch/chunk slices."""
    n = {}
    n["srcb"] = np.ascontiguousarray(src_b)
    n["srcq"] = np.ascontiguousarray(srcq)
    n["posq"] = np.ascontiguousarray(posq)
    n["refq"] = np.ascontiguousarray(refq.reshape(refq.shape[0], 8))
    # W_off columns reordered (h,l,p,2) -> (l,h,p,2); W_attn stays (h,l,p)
    woff = np.asarray(inputs["W_off"]).reshape(C, NH, NL, NP, 2)
    woff = woff.transpose(0, 2, 1, 3, 4).reshape(C, 256)
    boff = np.asarray(inputs["b_off"]).reshape(NH, NL, NP, 2)
    boff = boff.transpose(1, 0, 2, 3).reshape(256)
    n["wq"] = np.concatenate([woff, np.asarray(inputs["W_attn"])], axis=1)
    n["bq"] = np.concatenate([boff, np.asarray(inputs["b_attn"])])[None, :]
    n["wval"] = np.asarray(inputs["W_val"])
    n["bval"] = np.asarray(inputs["b_val"])[None, :]
    n["wout"] = np.asarray(inputs["W_out"])
    n["bout"] = np.asarray(inputs["b_out"])[None, :]
    n["w1"] = np.asarray(inputs["W1"])
    n["b1t"] = np.ascontiguousarray(np.asarray(inputs["b1"]).reshape(8, 128).T)
    n["w2"] = np.asarray(inputs["W2"])
    n["b2"] = np.asarray(inputs["b2"])[None, :]
    n["gb"] = np.stack([np.asarray(inputs["g1"]), np.asarray(inputs["beta1"]),
                        np.asarray(inputs["g2"]), np.asarray(inputs["beta2"])])
    consts = np.zeros((8, 128), np.float32)
    Wl = np.array([w for (h, w) in SHAPES], np.float32)
    Hl = np.array([h for (h, w) in SHAPES], np.float32)
    # rows in (l, h, p) order
    consts[0] = np.broadcast_to(Wl[:, None, None], (NL, NH, NP)).reshape(-1)
    consts[1] = np.broadcast_to((Wl - 2)[:, None, None], (NL, NH, NP)).reshape(-1)
    consts[2] = np.broadcast_to((Hl - 1)[:, None, None], (NL, NH, NP)).reshape(-1)
    consts[3] = np.broadcast_to((Hl - 2)[:, None, None], (NL, NH, NP)).reshape(-1)
    base = (np.array(LVL_START, np.float32)[:, None, None]
            + np.arange(NH)[None, :, None] * SPAD)
    consts[4] = np.broadcast_to(base, (NL, NH, NP)).reshape(-1).astype(np.float32)
    consts[5] = 1.0
    sc = np.zeros(128, np.float32)
    sc[0:8:2] = Wl
    sc[1:8:2] = Hl
    consts[6] = sc
    n["consts"] = consts
    n["ident"] = np.eye(128, dtype=np.float32)
    for k in n:
        n[k] = np.ascontiguousarray(n[k], dtype=np.float32)
    return n



_CACHE = {}


def _get_compiled():
    if "k" not in _CACHE:
        nc = bass.Bass()
        build(nc)
        _CACHE["k"] = CompiledKernel(nc, 8)
    return _CACHE["k"]


def _in_maps(inputs):
    src = np.asarray(inputs["src"], np.float32)
    pos = np.asarray(inputs["pos"], np.float32)
    ref = np.asarray(inputs["reference_points"], np.float32)
    N, S_, C_ = src.shape
    maps = []
    for c in range(8):
        n, k = c // 4, c % 4
        srcb = np.zeros((SPAD, C_), np.float32); srcb[:S_] = src[n]
        posb = np.zeros((SPAD, C_), np.float32); posb[:S_] = pos[n]
        refb = np.full((SPAD, 4, 2), 0.5, np.float32); refb[:S_] = ref[n]
        sl = slice(k * T, (k + 1) * T)
        maps.append(make_host_inputs(srcb, srcb[sl], posb[sl], refb[sl], inputs))
    return maps


def kernel(**inputs):
    k = _get_compiled()
    res = k.run(k.put(_in_maps(inputs)))
    out = np.zeros((2, SPAD, 256), np.float32)
    for c in range(8):
        n, kk = c // 4, c % 4
        out[n, kk * T:(kk + 1) * T] = res[c]["out"]
    return out[:, :S, :]



# revision 5
# speedup vs baseline: 1.1055x; 1.1055x over previous
import sys
sys.path.insert(0, "/opt/trn_rl_repo")
sys.path.insert(0, "/opt/trn_rl_repo/concourse")
"""nn_DeformableTransformerEncoderLayer_876173328776 on 8 trn2 NeuronCores.

kernel(**inputs) takes the FULL (unsharded) inputs and returns the FULL
[2, 13294, 256] float32 output.

Sharding: 8 cores = 2 batches x 4 sequence chunks of 3328 query tokens.
Each core recomputes the value tensor for its batch (replicated across that
batch's 4 cores; sampling offsets can reach anywhere in each level's feature
map), weights replicated.

Phase A builds a 2x2-patch table in DRAM: patches[h, s, slot, ch] f32 where
slot in {v[s], v[s+1], v[s+W_l], v[s+W_l+1]} -- one 512B row per bilinear
footprint.  Phase B samples with nc.gpsimd.dma_gather (the Ant bulk-gather
custom op, ~7.5us per 1024 rows vs ~78us per 128 rows for the generic
indirect-DMA path): 16 gathers of 1024 idxs per 128-token block.  grid_sample
zero-padding is folded into per-slot weights; DVE does the weighted combine;
FFN/LayerNorms fused on PE/ACT/DVE as before.
"""


import concourse.mybir as mybir

MAX_WAITS_PER_INST = 1


def split_waits(nc):
    """Post-process all functions of `nc` so no instruction carries more than
    MAX_WAITS_PER_INST sem-waits."""
    n_split = 0
    for fn in nc.m.functions:
        for bb in fn.blocks:
            insts = bb.instructions
            i = 0
            while i < len(insts):
                inst = insts[i]
                si = inst.sync_info
                if si is None or si.on_wait is None or len(si.on_wait) <= MAX_WAITS_PER_INST:
                    i += 1
                    continue
                waits = list(si.on_wait)
                si.on_wait = waits[-MAX_WAITS_PER_INST:]
                rest = waits[:-MAX_WAITS_PER_INST]
                nops = []
                for k in range(0, len(rest), MAX_WAITS_PER_INST):
                    nop = mybir.InstNoOp(
                        name=nc.get_next_instruction_name(),
                        engine=inst.engine,
                        ins=[],
                        outs=[],
                        sync_info=mybir.SyncInfo(
                            on_wait=rest[k : k + MAX_WAITS_PER_INST], on_update=[]
                        ),
                        text_hint="wait_split",
                    )
                    nc.register_instruction(nop, overwrite=True)
                    nops.append(nop)
                insts[i:i] = nops
                i += len(nops) + 1
                n_split += 1
    return n_split


def apply():
    """Monkeypatch TileContext.__exit__ to run split_waits after scheduling."""
    import concourse.tile as tile

    if getattr(tile.TileContext, "_wait_split_patched", False):
        return
    orig_exit = tile.TileContext.__exit__

    def patched_exit(self, exc_type, exc_value, tb):
        r = orig_exit(self, exc_type, exc_value, tb)
        if exc_type is None:
            split_waits(self.nc)
        return r

    tile.TileContext.__exit__ = patched_exit
    tile.TileContext._wait_split_patched = True



import time

import numpy as np
import jax
from jax.sharding import Mesh, PartitionSpec
from jax.experimental.shard_map import shard_map

import concourse.bass as bass
from concourse import bass2jax
from concourse.bass2jax import _bass_exec_p, install_neuronx_cc_hook, partition_id_tensor


def _install_loud_hook():
    """neuronx_cc hook that prints the real traceback on failure (the PJRT
    layer otherwise swallows it into an opaque INTERNAL error)."""
    import traceback
    import libneuronxla

    if getattr(libneuronxla, "_loud_hook_installed", False):
        return

    def hook(*a, **k):
        try:
            return bass2jax.neuronx_cc_hook(*a, **k)
        except BaseException:
            traceback.print_exc()
            raise

    if not hasattr(libneuronxla, "orig_neuronx_cc"):
        libneuronxla.orig_neuronx_cc = libneuronxla.neuronx_cc
    libneuronxla.neuronx_cc = hook
    libneuronxla._loud_hook_installed = True


class CompiledKernel:
    def __init__(self, nc: bass.Bass, n_cores: int):
        install_neuronx_cc_hook()
        _install_loud_hook()
        assert nc.dbg_addr is None or not nc.dbg_callbacks
        self.nc = nc
        self.n_cores = n_cores
        partition_name = nc.partition_id_tensor.name if nc.partition_id_tensor else None
        in_names, out_names, out_avals, zero_outs = [], [], [], []
        for alloc in nc.m.functions[0].allocations:
            if not isinstance(alloc, mybir.MemoryLocationSet):
                continue
            name = alloc.memorylocations[0].name
            if alloc.kind == "ExternalInput":
                if name != partition_name:
                    in_names.append(name)
            elif alloc.kind == "ExternalOutput":
                shape = tuple(alloc.tensor_shape)
                dtype = mybir.dt.np(alloc.dtype)
                out_names.append(name)
                out_avals.append(jax.core.ShapedArray(shape, dtype))
                zero_outs.append(np.zeros(shape, dtype))
        n_params = len(in_names)
        n_outs = len(out_avals)
        all_in_names = list(in_names) + list(out_names)
        if partition_name is not None:
            all_in_names.append(partition_name)
        self.in_names = in_names
        self.out_names = out_names
        self.out_avals = out_avals
        self.n_params = n_params

        def _body(*args):
            operands = list(args)
            if partition_name is not None:
                operands.append(partition_id_tensor())
            outs = _bass_exec_p.bind(
                *operands,
                out_avals=tuple(out_avals),
                in_names=tuple(all_in_names),
                out_names=tuple(out_names),
                lowering_input_output_aliases=(),
                sim_require_finite=False,
                sim_require_nnan=False,
                nc=nc,
            )
            return tuple(outs)

        donate = tuple(range(n_params, n_params + n_outs))
        if n_cores == 1:
            self._fn = jax.jit(_body, donate_argnums=donate, keep_unused=True)
            self.mesh = None
        else:
            devices = jax.devices()[:n_cores]
            self.mesh = Mesh(np.asarray(devices), ("core",))
            in_specs = (PartitionSpec("core"),) * (n_params + n_outs)
            out_specs = (PartitionSpec("core"),) * n_outs
            self._fn = jax.jit(
                shard_map(_body, mesh=self.mesh, in_specs=in_specs,
                          out_specs=out_specs, check_rep=False),
                donate_argnums=donate,
                keep_unused=True,
            )
        self._zero_outs = zero_outs

    def _prep(self, in_maps):
        assert len(in_maps) == self.n_cores
        if self.n_cores == 1:
            return [np.ascontiguousarray(in_maps[0][n]) for n in self.in_names]
        cat = []
        for i, n in enumerate(self.in_names):
            cat.append(np.concatenate(
                [np.ascontiguousarray(in_maps[c][n]) for c in range(self.n_cores)], axis=0))
        return cat

    def put(self, in_maps):
        """device_put inputs once; returns opaque handle for run()."""
        arrs = self._prep(in_maps)
        if self.n_cores == 1:
            return [jax.device_put(a, jax.devices()[0]) for a in arrs]
        from jax.sharding import NamedSharding
        sh = NamedSharding(self.mesh, PartitionSpec("core"))
        return [jax.device_put(a, sh) for a in arrs]

    def _zeros(self):
        if self.n_cores == 1:
            return [np.zeros(z.shape, z.dtype) for z in self._zero_outs]
        return [np.zeros((self.n_cores * z.shape[0], *z.shape[1:]), z.dtype)
                for z in self._zero_outs]

    def run(self, handle):
        outs = self._fn(*handle, *self._zeros())
        outs = [np.asarray(o) for o in outs]
        if self.n_cores == 1:
            return [dict(zip(self.out_names, outs))]
        res = []
        for c in range(self.n_cores):
            d = {}
            for i, n in enumerate(self.out_names):
                d[n] = outs[i].reshape(self.n_cores, *self.out_avals[i].shape)[c]
            res.append(d)
        return res

    def bench(self, handle, iters=5, warmup=2):
        for _ in range(warmup):
            outs = self._fn(*handle, *self._zeros())
            jax.block_until_ready(outs)
        ts = []
        for _ in range(iters):
            t0 = time.perf_counter()
            outs = self._fn(*handle, *self._zeros())
            jax.block_until_ready(outs)
            ts.append(time.perf_counter() - t0)
        return min(ts), ts


apply()

import concourse.tile as tile
from concourse import library_config
from concourse.library_overlay import lower_extended_insts

F32 = mybir.dt.float32
I32 = mybir.dt.int32
I16 = mybir.dt.int16
AX = mybir.AxisListType
OP = mybir.AluOpType
AF = mybir.ActivationFunctionType

SHAPES = [(100, 100), (50, 50), (25, 25), (13, 13)]
NH, NL, NP, C, DF, HD = 8, 4, 4, 256, 1024, 32
S = 13294
SPAD = 13312          # padded batch tokens (104 * 128)
T = 3328              # per-core query tokens (26 * 128)
NBLK_A = SPAD // 128  # 104
NBLK_B = T // 128     # 26
LVL_START = [0, 10000, 12500, 13125]
NIDX = 1024           # idxs per dma_gather (SWDGE ring cap is < 2048)


def build(nc: bass.Bass):
    # ---------------- I/O ----------------
    srcb = nc.dram_tensor("srcb", [SPAD, C], F32, kind="ExternalInput")
    srcq = nc.dram_tensor("srcq", [T, C], F32, kind="ExternalInput")
    posq = nc.dram_tensor("posq", [T, C], F32, kind="ExternalInput")
    refq = nc.dram_tensor("refq", [T, 8], F32, kind="ExternalInput")
    wq = nc.dram_tensor("wq", [C, 384], F32, kind="ExternalInput")
    bq = nc.dram_tensor("bq", [1, 384], F32, kind="ExternalInput")
    wval = nc.dram_tensor("wval", [C, C], F32, kind="ExternalInput")
    bval = nc.dram_tensor("bval", [1, C], F32, kind="ExternalInput")
    wout = nc.dram_tensor("wout", [C, C], F32, kind="ExternalInput")
    bout = nc.dram_tensor("bout", [1, C], F32, kind="ExternalInput")
    w1 = nc.dram_tensor("w1", [C, DF], F32, kind="ExternalInput")
    b1t = nc.dram_tensor("b1t", [128, 8], F32, kind="ExternalInput")
    w2 = nc.dram_tensor("w2", [DF, C], F32, kind="ExternalInput")
    b2 = nc.dram_tensor("b2", [1, C], F32, kind="ExternalInput")
    gb = nc.dram_tensor("gb", [4, C], F32, kind="ExternalInput")
    # consts rows (each [128] in (h,l,p) order unless noted):
    # 0: W_l  1: W_l-2  2: H_l-2  3: lvl_start + (h%2)*SPAD  4: ones
    # 5: (l,xy)-interleaved scale row [W0,H0,W1,H1,...] (first 8 cols)
    consts = nc.dram_tensor("consts", [8, 128], F32, kind="ExternalInput")
    ident = nc.dram_tensor("ident", [128, 128], F32, kind="ExternalInput")
    # selrep[j*128 + t, o] = 1 iff t == 16j + o%16  (idx shuffle matmuls)
    selrep = nc.dram_tensor("selrep", [8 * 128, 128], F32, kind="ExternalInput")
    out = nc.dram_tensor("out", [T, C], F32, kind="ExternalOutput")

    # patch table: patches[h, s] = [v[s], v[s+1], v[s+W_l], v[s+W_l+1]] (f32)
    patches = nc.dram_tensor("patches", [NH, SPAD, 4, 32], F32, kind="Internal")

    nreg_cm = nc.gpsimd.register("nidx")

    with tile.TileContext(nc) as tc:
        # ------------- persistent weights/constants -------------
        with tc.tile_pool(name="wpool", bufs=1) as wp:
            nc.gpsimd.load_library(library_config.mlp)
            nreg = nreg_cm.__enter__()
            nc.gpsimd.reg_mov(nreg, NIDX)

            wq_sb = [wp.tile([128, 384], F32, name=f"wq{k}", tag=f"wq{k}") for k in range(2)]
            wval_sb = [wp.tile([128, C], F32, name=f"wval{k}", tag=f"wval{k}") for k in range(2)]
            wout_sb = [wp.tile([128, C], F32, name=f"wout{k}", tag=f"wout{k}") for k in range(2)]
            w1_sb = [wp.tile([128, DF], F32, name=f"w1{k}", tag=f"w1{k}") for k in range(2)]
            w2_sb = [wp.tile([128, C], F32, name=f"w2{k}", tag=f"w2{k}") for k in range(8)]
            for k in range(2):
                nc.sync.dma_start(wq_sb[k][:], wq[128 * k:128 * (k + 1), :])
                nc.sync.dma_start(wval_sb[k][:], wval[128 * k:128 * (k + 1), :])
                nc.sync.dma_start(wout_sb[k][:], wout[128 * k:128 * (k + 1), :])
                nc.sync.dma_start(w1_sb[k][:], w1[128 * k:128 * (k + 1), :])
            for k in range(8):
                nc.sync.dma_start(w2_sb[k][:], w2[128 * k:128 * (k + 1), :])
            bq_sb = wp.tile([1, 384], F32)
            bval_sb = wp.tile([1, C], F32)
            bout_sb = wp.tile([1, C], F32)
            b2_sb = wp.tile([1, C], F32)
            b1t_sb = wp.tile([128, 8], F32)
            nc.sync.dma_start(bq_sb[:], bq[:])
            nc.sync.dma_start(bval_sb[:], bval[:])
            nc.sync.dma_start(bout_sb[:], bout[:])
            nc.sync.dma_start(b2_sb[:], b2[:])
            nc.sync.dma_start(b1t_sb[:], b1t[:])
            gbt = [wp.tile([128, C], F32, name=f"gb{i}", tag=f"gb{i}") for i in range(4)]
            for i in range(4):
                nc.sync.dma_start(gbt[i][:], gb[i:i + 1, :].to_broadcast([128, C]))
            ident_sb = wp.tile([128, 128], F32)
            nc.sync.dma_start(ident_sb[:], ident[:])
            ones_sb = wp.tile([1, 128], F32)
            nc.sync.dma_start(ones_sb[:], consts[4:5, :])
            sel_sb = [wp.tile([128, 128], F32, name=f"sel{j}", tag=f"sel{j}") for j in range(8)]
            for j in range(8):
                nc.sync.dma_start(sel_sb[j][:], selrep[128 * j:128 * (j + 1), :])
            # broadcast const rows to [128,128]
            cWl = wp.tile([128, 128], F32)
            cWm2 = wp.tile([128, 128], F32)
            cHm2 = wp.tile([128, 128], F32)
            cBase = wp.tile([128, 128], F32)
            for t_, row in ((cWl, 0), (cWm2, 1), (cHm2, 2), (cBase, 3)):
                nc.sync.dma_start(t_[:], consts[row:row + 1, :].to_broadcast([128, 128]))
            cScale = wp.tile([128, 8], F32)
            nc.sync.dma_start(cScale[:], consts[5:6, 0:8].to_broadcast([128, 8]))

            # ---------------- Phase A: value -> patch table ----------------
            # per level: slot shifts {0, 1, W_l, W_l+1}; patch rows of level l
            # live in [base_l, base_l + H*W); only rows reachable from clipped
            # sample coords are ever gathered, so slot writes can stay within
            # each level's source range.
            lvl_rng = [(LVL_START[l], LVL_START[l] + h * w)
                       for l, (h, w) in enumerate(SHAPES)]
            # zero the pad rows (never gathered, but keeps DRAM finite)
            ztile = wp.tile([128, 1024], F32, name="zz", tag="zz")
            nc.vector.memset(ztile[:], 0.0)
            nc.sync.dma_start(
                patches[:, S:SPAD, :, :].rearrange("h s r c -> s h (r c)"),
                ztile[0:SPAD - S, :].rearrange("s (h x) -> s h x", h=NH))
            with tc.tile_pool(name="pa", bufs=3) as pa, \
                 tc.tile_pool(name="pa_ps", bufs=2, space="PSUM") as pa_ps:
                for b in range(NBLK_A):
                    s0, s1 = b * 128, (b + 1) * 128
                    sb_s = pa.tile([128, C], F32, name="sb_s", tag="sb_s")
                    nc.sync.dma_start(sb_s[:], srcb[s0:s1, :])
                    sT = [pa.tile([128, 128], F32, name=f"sT{k}", tag=f"sT{k}") for k in range(2)]
                    for k in range(2):
                        ps_t = pa_ps.tile([128, 128], F32, name="ps_t", tag="ps_t")
                        nc.tensor.transpose(ps_t[:], sb_s[:, 128 * k:128 * (k + 1)],
                                            ident_sb[:])
                        nc.vector.tensor_copy(sT[k][:], ps_t[:])
                    ps_v = pa_ps.tile([128, C], F32, name="ps_v", tag="ps_v")
                    nc.tensor.matmul(ps_v[:], sT[0][:], wval_sb[0][:],
                                     start=True, stop=False)
                    nc.tensor.matmul(ps_v[:], sT[1][:], wval_sb[1][:],
                                     start=False, stop=False)
                    nc.tensor.matmul(ps_v[:], ones_sb[:], bval_sb[:],
                                     start=False, stop=True)
                    sb_v = pa.tile([128, C], F32, name="sb_v", tag="sb_v")
                    nc.scalar.copy(sb_v[:], ps_v[:])
                    sb_v3 = sb_v[:].rearrange("s (h c) -> s h c", h=NH)

                    def slot_write(shift, slot, lo, hi):
                        # patch rows p in [lo, hi): patches[:, p, slot] = v[p+shift]
                        p0 = max(lo, s0 - shift)
                        p1 = min(hi, s1 - shift)
                        if p0 >= p1:
                            return
                        t0 = p0 + shift - s0
                        nc.sync.dma_start(
                            patches[:, p0:p1, slot, :].rearrange("h s c -> s h c"),
                            sb_v3[t0:t0 + (p1 - p0)])

                    slot_write(0, 0, 0, S)
                    slot_write(1, 1, 0, S)
                    for l, (hh, ww) in enumerate(SHAPES):
                        lo, hi = lvl_rng[l]
                        slot_write(ww, 2, lo, hi)
                        slot_write(ww + 1, 3, lo, hi)

            # ---------------- Phase B ----------------
            with tc.tile_pool(name="pb", bufs=2) as pb, \
                 tc.tile_pool(name="sc", bufs=2) as sc, \
                 tc.tile_pool(name="gat", bufs=3) as gat, \
                 tc.tile_pool(name="wvp", bufs=2) as wvp, \
                 tc.tile_pool(name="pb_ps", bufs=2, space="PSUM") as pbps:
                for b in range(NBLK_B):
                    tok = slice(b * 128, (b + 1) * 128)
                    sb_src = pb.tile([128, C], F32, name="sb_src", tag="sb_src")
                    sb_q = pb.tile([128, C], F32, name="sb_q", tag="sb_q")
                    sb_ref = pb.tile([128, 8], F32, name="sb_ref", tag="sb_ref")
                    nc.sync.dma_start(sb_src[:], srcq[tok, :])
                    nc.sync.dma_start(sb_q[:], posq[tok, :])
                    nc.sync.dma_start(sb_ref[:], refq[tok, :])
                    nc.vector.tensor_add(sb_q[:], sb_q[:], sb_src[:])
                    # q^T
                    qT = [pb.tile([128, 128], F32, name=f"qT{k}", tag=f"qT{k}") for k in range(2)]
                    for k in range(2):
                        ps_t = pbps.tile([128, 128], F32, name="ps_t", tag="ps_t")
                        nc.tensor.transpose(ps_t[:], sb_q[:, 128 * k:128 * (k + 1)],
                                            ident_sb[:])
                        nc.vector.tensor_copy(qT[k][:], ps_t[:])
                    # off|logits   (native (h,l,p) column order)
                    ps_q = pbps.tile([128, 384], F32, name="ps_q", tag="ps_q", bufs=1)
                    nc.tensor.matmul(ps_q[:], qT[0][:], wq_sb[0][:],
                                     start=True, stop=False)
                    nc.tensor.matmul(ps_q[:], qT[1][:], wq_sb[1][:],
                                     start=False, stop=False)
                    nc.tensor.matmul(ps_q[:], ones_sb[:], bq_sb[:],
                                     start=False, stop=True)

                    # softmax over 16 per head  (logits at [:,256:384], (h,l,p))
                    logit = ps_q[:, 256:384].rearrange("p (h s) -> p h s", h=NH)
                    rmax = sc.tile([128, 8], F32, name="rmax", tag="rmax")
                    nc.vector.reduce_max(rmax[:], logit, axis=AX.X)
                    sb_e = sc.tile([128, 128], F32, name="sb_e", tag="sb_e")
                    nc.vector.scalar_tensor_tensor(
                        out=sb_e[:].rearrange("p (h s) -> p h s", h=NH),
                        in0=logit, scalar=1.0,
                        in1=rmax[:].to_broadcast([128, NH, 16]),
                        op0=OP.bypass, op1=OP.subtract)
                    nc.scalar.activation(sb_e[:], sb_e[:], AF.Exp)
                    rsum = sc.tile([128, 8], F32, name="rsum", tag="rsum")
                    nc.vector.reduce_sum(rsum[:], sb_e[:].rearrange(
                        "p (h s) -> p h s", h=NH), axis=AX.X)
                    nc.vector.reciprocal(rsum[:], rsum[:])
                    sb_aw = sc.tile([128, 128], F32, name="sb_aw", tag="sb_aw")
                    nc.vector.tensor_mul(
                        sb_aw[:].rearrange("p (h s) -> p h s", h=NH),
                        sb_e[:].rearrange("p (h s) -> p h s", h=NH),
                        rsum[:].to_broadcast([128, NH, 16]))

                    # ---- sampling coordinates ----  (physical (h,l,p) order)
                    rs = sc.tile([128, 8], F32, name="rs", tag="rs")
                    nc.vector.tensor_mul(rs[:], sb_ref[:], cScale[:])
                    nc.vector.tensor_scalar_add(rs[:], rs[:], -0.5)
                    rsv = rs[:].rearrange("p (l two) -> p l two", l=NL)
                    # (l,h,p)-ordered strided views over (h,l,p) storage
                    offv = ps_q[:, 0:256].rearrange(
                        "p (h l pp two) -> p l h pp two", h=NH, l=NL, pp=NP)
                    X = sc.tile([128, 128], F32, name="X", tag="X")
                    Y = sc.tile([128, 128], F32, name="Y", tag="Y")
                    lhp_v = lambda ap: ap.rearrange("p (h l pp) -> p l h pp",
                                                    h=NH, l=NL)
                    nc.vector.tensor_add(
                        lhp_v(X[:]), offv[:, :, :, :, 0],
                        rsv[:, :, 0].to_broadcast([128, NL, NH, NP]))
                    nc.vector.tensor_add(
                        lhp_v(Y[:]), offv[:, :, :, :, 1],
                        rsv[:, :, 1].to_broadcast([128, NL, NH, NP]))

                    def floor_(dst_tag, src):
                        ti = sc.tile([128, 128], I32, tag=dst_tag + "i")
                        tf = sc.tile([128, 128], F32, tag=dst_tag)
                        nc.vector.tensor_copy(ti[:], src[:])
                        nc.vector.tensor_copy(tf[:], ti[:])
                        gt = sc.tile([128, 128], F32, tag=dst_tag + "g")
                        nc.vector.tensor_tensor(gt[:], tf[:], src[:], op=OP.is_gt)
                        nc.vector.tensor_tensor(tf[:], tf[:], gt[:], op=OP.subtract)
                        return tf

                    X0 = floor_("X0", X)
                    Y0 = floor_("Y0", Y)
                    fx = sc.tile([128, 128], F32, name="fx", tag="fx")
                    fy = sc.tile([128, 128], F32, name="fy", tag="fy")
                    nc.vector.tensor_tensor(fx[:], X[:], X0[:], op=OP.subtract)
                    nc.vector.tensor_tensor(fy[:], Y[:], Y0[:], op=OP.subtract)

                    tt = sc.tile([128, 128], F32, name="tt", tag="tt")

                    def edge_weights(C0, fc, cMax, tagp):
                        # cs = clip(C0, 0, max); d = cs - C0;
                        # wA = (1-fc)*[d==0] + fc*[d==1]
                        # wB = (1-fc)*[d==-1] + fc*[d==0]
                        cs = sc.tile([128, 128], F32, name=tagp + "cs", tag=tagp + "cs")
                        nc.vector.tensor_scalar_max(cs[:], C0[:], 0.0)
                        nc.vector.tensor_tensor(cs[:], cs[:], cMax[:], op=OP.min)
                        d = sc.tile([128, 128], F32, name=tagp + "d", tag=tagp + "d")
                        nc.vector.tensor_tensor(d[:], cs[:], C0[:], op=OP.subtract)
                        eq0 = sc.tile([128, 128], F32, tag=tagp + "eq0")
                        eq1 = sc.tile([128, 128], F32, tag=tagp + "eq1")
                        eqm1 = sc.tile([128, 128], F32, tag=tagp + "eqm1")
                        nc.vector.tensor_scalar(eq0[:], d[:], 0.0, None, op0=OP.is_equal)
                        nc.vector.tensor_scalar(eq1[:], d[:], 1.0, None, op0=OP.is_equal)
                        nc.vector.tensor_scalar(eqm1[:], d[:], -1.0, None, op0=OP.is_equal)
                        w0 = sc.tile([128, 128], F32, tag=tagp + "w0")
                        nc.vector.tensor_scalar(w0[:], fc[:], -1.0, 1.0,
                                                op0=OP.mult, op1=OP.add)
                        wA = sc.tile([128, 128], F32, tag=tagp + "wA")
                        wB = sc.tile([128, 128], F32, tag=tagp + "wB")
                        nc.vector.tensor_mul(wA[:], w0[:], eq0[:])
                        nc.vector.tensor_mul(tt[:], fc[:], eq1[:])
                        nc.vector.tensor_add(wA[:], wA[:], tt[:])
                        nc.vector.tensor_mul(wB[:], w0[:], eqm1[:])
                        nc.vector.tensor_mul(tt[:], fc[:], eq0[:])
                        nc.vector.tensor_add(wB[:], wB[:], tt[:])
                        return cs, wA, wB

                    xs, wxA, wxB = edge_weights(X0, fx, cWm2, "x")
                    ys, wyA, wyB = edge_weights(Y0, fy, cHm2, "y")

                    awy0 = sc.tile([128, 128], F32, name="awy0", tag="awy0")
                    awy1 = sc.tile([128, 128], F32, name="awy1", tag="awy1")
                    nc.vector.tensor_mul(awy0[:], sb_aw[:], wyA[:])
                    nc.vector.tensor_mul(awy1[:], sb_aw[:], wyB[:])
                    # W assembly [128, (h l p r x)=512]
                    Wt = sc.tile([128, 512], F32, name="Wt", tag="Wt")
                    Wv = Wt[:].rearrange("p (q r x) -> p q r x", r=2, x=2)
                    nc.vector.tensor_mul(Wv[:, :, 0, 0], awy0[:], wxA[:])
                    nc.vector.tensor_mul(Wv[:, :, 0, 1], awy0[:], wxB[:])
                    nc.vector.tensor_mul(Wv[:, :, 1, 0], awy1[:], wxA[:])
                    nc.vector.tensor_mul(Wv[:, :, 1, 1], awy1[:], wxB[:])

                    # ---- patch index [t, (h l p)] -> shuffled idx tile ----
                    idxf = sc.tile([128, 128], F32, name="idxf", tag="idxf")
                    nc.vector.tensor_add(tt[:], xs[:], cBase[:])
                    nc.vector.tensor_mul(idxf[:], ys[:], cWl[:])
                    nc.vector.tensor_add(idxf[:], idxf[:], tt[:])
                    # idx_i16[r(+16g), c*8+j] = idxf[16j+r, c]
                    idx_i16 = sc.tile([128, 1024], I16, name="idx16", tag="idx16")
                    idx_v = idx_i16[:].rearrange("p (c j) -> p c j", j=8)
                    for j in range(8):
                        ps_s = pbps.tile([128, 128], F32, name="ps_s", tag="ps_t")
                        nc.tensor.matmul(ps_s[:], sel_sb[j][:], idxf[:],
                                         start=True, stop=True)
                        nc.vector.tensor_copy(idx_v[:, :, j], ps_s[:])

                    # ---- gather + combine ----
                    attn = sc.tile([128, 256], F32, name="attn", tag="attn")
                    for g in range(16):
                        h = g // 2
                        p2 = h // 2
                        tbl = patches[2 * p2:2 * p2 + 2].rearrange(
                            "h s r c -> (h s) (r c)")
                        vt = gat.tile([128, 8, 128], F32, name="vt", tag="vt")
                        nc.gpsimd.dma_gather(
                            vt[:], tbl, idx_i16[:, g * 64:(g + 1) * 64],
                            num_idxs=NIDX, num_idxs_reg=nreg,
                            elem_size=128, elem_step=128)
                        wv = wvp.tile([128, 8, 4, 32], F32, name="wv", tag="wv")
                        nc.vector.tensor_mul(
                            wv[:],
                            vt[:].rearrange("p c (r ch) -> p c r ch", r=4),
                            Wt[:, g * 32:(g + 1) * 32]
                            .rearrange("p (c r) -> p c r", r=4)
                            .to_broadcast([128, 8, 4, 32]))
                        red_view = wv[:].rearrange("p c r ch -> p ch (c r)")
                        if g % 2 == 0:
                            nc.vector.reduce_sum(
                                attn[:, h * 32:(h + 1) * 32], red_view, axis=AX.X)
                        else:
                            red = sc.tile([128, 32], F32, name="red", tag="red")
                            nc.vector.reduce_sum(red[:], red_view, axis=AX.X)
                            nc.vector.tensor_add(
                                attn[:, h * 32:(h + 1) * 32],
                                attn[:, h * 32:(h + 1) * 32], red[:])

                    # ---- src2 = attn @ Wout + bout ----
                    aT = [pb.tile([128, 128], F32, name=f"aT{k}", tag=f"aT{k}") for k in range(2)]
                    for k in range(2):
                        ps_t = pbps.tile([128, 128], F32, name="ps_t", tag="ps_t")
                        nc.tensor.transpose(ps_t[:], attn[:, 128 * k:128 * (k + 1)],
                                            ident_sb[:])
                        nc.vector.tensor_copy(aT[k][:], ps_t[:])
                    ps_o = pbps.tile([128, C], F32, name="ps_o", tag="ps_o", bufs=1)
                    nc.tensor.matmul(ps_o[:], aT[0][:], wout_sb[0][:],
                                     start=True, stop=False)
                    nc.tensor.matmul(ps_o[:], aT[1][:], wout_sb[1][:],
                                     start=False, stop=False)
                    nc.tensor.matmul(ps_o[:], ones_sb[:], bout_sb[:],
                                     start=False, stop=True)

                    # ---- x = LN1(src + src2) ----
                    def layer_norm(ps_in, resid, g_t, b_t, out_tile):
                        r = sc.tile([128, C], F32, name="ln_r", tag="ln_r")
                        msum = sc.tile([128, 1], F32, name="ln_m", tag="ln_m")
                        nc.vector.scalar_tensor_tensor(
                            out=r[:], in0=ps_in, scalar=1.0, in1=resid,
                            op0=OP.bypass, op1=OP.add, accum_out=msum[:])
                        nc.vector.tensor_scalar_mul(msum[:], msum[:], -1.0 / C)
                        xc = sc.tile([128, C], F32, name="ln_xc", tag="ln_xc")
                        nc.vector.tensor_scalar_add(xc[:], r[:], msum[:])
                        sq = sc.tile([128, C], F32, name="ln_sq", tag="ln_sq")
                        vsum = sc.tile([128, 1], F32, name="ln_v", tag="ln_v")
                        nc.scalar.activation(sq[:], xc[:], AF.Square,
                                             accum_out=vsum[:])
                        nc.vector.tensor_scalar(vsum[:], vsum[:], 1.0 / C, 1e-5,
                                                op0=OP.mult, op1=OP.add)
                        nc.scalar.sqrt(vsum[:], vsum[:])
                        nc.vector.reciprocal(vsum[:], vsum[:])
                        nc.vector.scalar_tensor_tensor(
                            out=out_tile, in0=xc[:], scalar=vsum[:], in1=g_t,
                            op0=OP.mult, op1=OP.mult)
                        nc.vector.tensor_add(out_tile, out_tile, b_t)
                        return out_tile

                    sb_x = pb.tile([128, C], F32, name="sb_x", tag="sb_x")
                    layer_norm(ps_o[:], sb_src[:], gbt[0][:], gbt[1][:], sb_x[:])

                    # ---- FFN ----
                    xT = [pb.tile([128, 128], F32, name=f"xT{k}", tag=f"xT{k}") for k in range(2)]
                    for k in range(2):
                        ps_t = pbps.tile([128, 128], F32, name="ps_t", tag="ps_t")
                        nc.tensor.transpose(ps_t[:], sb_x[:, 128 * k:128 * (k + 1)],
                                            ident_sb[:])
                        nc.vector.tensor_copy(xT[k][:], ps_t[:])
                    h1 = [pb.tile([128, 128], F32, name=f"h1_{m}", tag=f"h1_{m}") for m in range(8)]
                    for m in range(8):
                        ps_h = pbps.tile([128, 128], F32, name="ps_h", tag="ps_h")
                        nc.tensor.matmul(ps_h[:], w1_sb[0][:, 128 * m:128 * (m + 1)],
                                         xT[0][:], start=True, stop=False)
                        nc.tensor.matmul(ps_h[:], w1_sb[1][:, 128 * m:128 * (m + 1)],
                                         xT[1][:], start=False, stop=True)
                        nc.scalar.activation(h1[m][:], ps_h[:], AF.Relu,
                                             bias=b1t_sb[:, m:m + 1])
                    ps_f = pbps.tile([128, C], F32, name="ps_f", tag="ps_f", bufs=1)
                    for m in range(8):
                        nc.tensor.matmul(ps_f[:], h1[m][:], w2_sb[m][:],
                                         start=(m == 0), stop=False)
                    nc.tensor.matmul(ps_f[:], ones_sb[:], b2_sb[:],
                                     start=False, stop=True)
                    sb_out = pb.tile([128, C], F32, name="sb_out", tag="sb_out")
                    layer_norm(ps_f[:], sb_x[:], gbt[2][:], gbt[3][:], sb_out[:])
                    nc.sync.dma_start(out[tok, :], sb_out[:])
    return nc


def make_host_inputs(src_b, srcq, posq, refq, inputs):
    """Build the per-core input map (numpy) given the batch/chunk slices."""
    n = {}
    n["srcb"] = np.ascontiguousarray(src_b)
    n["srcq"] = np.ascontiguousarray(srcq)
    n["posq"] = np.ascontiguousarray(posq)
    n["refq"] = np.ascontiguousarray(refq.reshape(refq.shape[0], 8))
    # W_off / W_attn native (h,l,p[,2]) column order
    n["wq"] = np.concatenate([np.asarray(inputs["W_off"]),
                              np.asarray(inputs["W_attn"])], axis=1)
    n["bq"] = np.concatenate([np.asarray(inputs["b_off"]),
                              np.asarray(inputs["b_attn"])])[None, :]
    n["wval"] = np.asarray(inputs["W_val"])
    n["bval"] = np.asarray(inputs["b_val"])[None, :]
    n["wout"] = np.asarray(inputs["W_out"])
    n["bout"] = np.asarray(inputs["b_out"])[None, :]
    n["w1"] = np.asarray(inputs["W1"])
    n["b1t"] = np.ascontiguousarray(np.asarray(inputs["b1"]).reshape(8, 128).T)
    n["w2"] = np.asarray(inputs["W2"])
    n["b2"] = np.asarray(inputs["b2"])[None, :]
    n["gb"] = np.stack([np.asarray(inputs["g1"]), np.asarray(inputs["beta1"]),
                        np.asarray(inputs["g2"]), np.asarray(inputs["beta2"])])
    consts = np.zeros((8, 128), np.float32)
    Wl = np.array([w for (h, w) in SHAPES], np.float32)
    Hl = np.array([h for (h, w) in SHAPES], np.float32)
    # rows in (h, l, p) order
    hl = np.broadcast_to(Wl[None, :, None], (NH, NL, NP))
    consts[0] = hl.reshape(-1)
    consts[1] = np.broadcast_to((Wl - 2)[None, :, None], (NH, NL, NP)).reshape(-1)
    consts[2] = np.broadcast_to((Hl - 2)[None, :, None], (NH, NL, NP)).reshape(-1)
    base = (np.array(LVL_START, np.float32)[None, :, None]
            + (np.arange(NH) % 2)[:, None, None] * SPAD)
    consts[3] = np.broadcast_to(base, (NH, NL, NP)).reshape(-1).astype(np.float32)
    consts[4] = 1.0
    sc = np.zeros(128, np.float32)
    sc[0:8:2] = Wl
    sc[1:8:2] = Hl
    consts[5] = sc
    n["consts"] = consts
    n["ident"] = np.eye(128, dtype=np.float32)
    selrep = np.zeros((8 * 128, 128), np.float32)
    for j in range(8):
        for o in range(128):
            selrep[j * 128 + 16 * j + (o % 16), o] = 1.0
    n["selrep"] = selrep
    for k in n:
        n[k] = np.ascontiguousarray(n[k], dtype=np.float32)
    return n



_CACHE = {}


def _get_compiled():
    if "k" not in _CACHE:
        nc = bass.Bass()
        build(nc)
        lower_extended_insts(nc)
        _CACHE["k"] = CompiledKernel(nc, 8)
    return _CACHE["k"]


def _in_maps(inputs):
    src = np.asarray(inputs["src"], np.float32)
    pos = np.asarray(inputs["pos"], np.float32)
    ref = np.asarray(inputs["reference_points"], np.float32)
    N, S_, C_ = src.shape
    maps = []
    for c in range(8):
        n, k = c // 4, c % 4
        srcb = np.zeros((SPAD, C_), np.float32); srcb[:S_] = src[n]
        posb = np.zeros((SPAD, C_), np.float32); posb[:S_] = pos[n]
        refb = np.full((SPAD, 4, 2), 0.5, np.float32); refb[:S_] = ref[n]
        sl = slice(k * T, (k + 1) * T)
        maps.append(make_host_inputs(srcb, srcb[sl], posb[sl], refb[sl], inputs))
    return maps


def kernel(**inputs):
    k = _get_compiled()
    res = k.run(k.put(_in_maps(inputs)))
    out = np.zeros((2, SPAD, 256), np.float32)
    for c in range(8):
        n, kk = c // 4, c % 4
        out[n, kk * T:(kk + 1) * T] = res[c]["out"]
    return out[:, :S, :]
